# revision 7
# baseline (speedup 1.0000x reference)
"""Trainium2 Bass kernel for nn_ConnectLoss (pairwise BCE-Dice instance loss).

Strategy (8 NeuronCores, pixel-sharded):
  - Each core gets H/8 = 256 rows (524288 pixels) of all four inputs,
    viewed as [128 partitions, 4096 free].
  - Heavy part is the joint histogram inter[N=16, K=32] between target/pred
    instance labels. Per core: build fp16 one-hot indicator tiles on DVE
    (tensor_scalar is_equal, 4x mode), then contract on the TensorE with
    J=8 pixel-columns packed per matmul: stationary = [128, 8*15] strided
    view of target one-hots (classes 1..15), moving = [128, 8*34]
    (31 pred one-hots + cls, ln(cls), ln(1-cls)), output accumulated in a
    [120, 272] PSUM tile across all 512 groups. Only the 8 diagonal
    [15, 34] blocks are meaningful; host sums them.
  - Class-0 rows/cols and marginals are reconstructed exactly on the host
    from per-partition accum_out sums of the is_equal ops (free on DVE).
  - sum(pred_score^2) and the cls_out log-sums via ACT accum_out.

cls_out is uniform in [1e-4, 1-1e-4] so the torch-style -100 log clamp can
never trigger; logs are computed unclamped. cls stays fp32 into ACT so
ln(1-cls) keeps precision near cls ~ 1.
"""

import sys

if "/opt/trn_rl_repo" not in sys.path:
    sys.path.insert(0, "/opt/trn_rl_repo")

import numpy as np
from contextlib import ExitStack

# ---------------------------------------------------------------- constants
P = 128
H, W = 2048, 2048
NCORES = 8
ROWS = H // NCORES                 # 256 rows per core
PIX = ROWS * W                     # 524288 pixels per core
FPP = PIX // P                     # 4096 free elems per partition
CHUNKS = [896, 896, 896, 896, 512]
assert sum(CHUNKS) == FPP
NCHUNK = len(CHUNKS)
J = 8                              # pixel-columns packed per matmul
K = 32                             # pred instance classes
KB = K - 1                         # pred one-hot classes actually built
N = 16                             # target instance classes
NT = N - 1                         # target one-hot classes built (1..15)
MCOL = KB + 3                      # 34 moving cols: poh + cls, ln, ln1m
SROW = NT * J                      # 120 psum partitions
SCOL = MCOL * J                    # 272 psum free size

# output packing: [128, OUTC] f32
OC_PAIR = 0                        # rows 0:120, cols 0:272
OC_SP = SCOL                       # [128, KB*NCHUNK]
OC_ST = OC_SP + KB * NCHUNK        # [128, NT*NCHUNK]
OC_AUX = OC_ST + NT * NCHUNK       # [128, 4*NCHUNK]
OUTC = OC_AUX + 4 * NCHUNK

SMOOTH = 1.0
HWPIX = float(H * W)

_cached = {}


def _build_bass():
    import concourse.bass as bass
    import concourse.bacc as bacc
    import concourse.mybir as mybir
    from concourse.tile import TileContext

    f32 = mybir.dt.float32
    f16 = mybir.dt.float16
    i32 = mybir.dt.int32
    eq = mybir.AluOpType.is_equal
    add = mybir.AluOpType.add
    AF = mybir.ActivationFunctionType

    nc = bacc.Bacc("TRN2", num_swdge_queues=4)
    pm_d = nc.dram_tensor("pm", [PIX], i32, kind="ExternalInput")
    tm_d = nc.dram_tensor("tm", [PIX], i32, kind="ExternalInput")
    cls_d = nc.dram_tensor("cls", [PIX], f32, kind="ExternalInput")
    ps_d = nc.dram_tensor("ps", [PIX], f32, kind="ExternalInput")
    out_d = nc.dram_tensor("out", [P, OUTC], f32, kind="ExternalOutput")

    pm_v = pm_d[:].rearrange("(p f) -> p f", p=P)
    tm_v = tm_d[:].rearrange("(p f) -> p f", p=P)
    cls_v = cls_d[:].rearrange("(p f) -> p f", p=P)
    ps_v = ps_d[:].rearrange("(p f) -> p f", p=P)

    with ExitStack() as es:
        tc = es.enter_context(TileContext(nc))
        pool_in = es.enter_context(tc.tile_pool(name="inp", bufs=2))
        pool_toh = es.enter_context(tc.tile_pool(name="toh", bufs=2))
        pool_mov = es.enter_context(tc.tile_pool(name="mov", bufs=2))
        pool_misc = es.enter_context(tc.tile_pool(name="misc", bufs=1))
        pool_scr = es.enter_context(tc.tile_pool(name="scr", bufs=2))
        psum = es.enter_context(tc.tile_pool(name="ps", bufs=1, space="PSUM"))

        pair_ps = psum.tile([SROW, SCOL], f32)
        spacc = pool_misc.tile([P, KB * NCHUNK], f32)
        stacc = pool_misc.tile([P, NT * NCHUNK], f32)
        auxacc = pool_misc.tile([P, 4 * NCHUNK], f32)

        off = 0
        for c, CF in enumerate(CHUNKS):
            cs = slice(off, off + CF)
            # labels arrive as fp16 via casting DMA (gpsimd SWDGE)
            pm16 = pool_in.tile([P, CF], f16, tag="pm16")
            tm16 = pool_in.tile([P, CF], f16, tag="tm16")
            cls_t = pool_in.tile([P, CF], f32, tag="cls")
            ps16 = pool_in.tile([P, CF], f16, tag="ps")
            nc.gpsimd.dma_start(out=pm16[:], in_=pm_v[:, cs])
            nc.gpsimd.dma_start(out=tm16[:], in_=tm_v[:, cs])
            nc.sync.dma_start(out=cls_t[:], in_=cls_v[:, cs])
            nc.gpsimd.dma_start(out=ps16[:], in_=ps_v[:, cs])

            # target one-hots, classes 1..15, x-major interleaved so each
            # matmul group's stationary block [128, 120] is contiguous
            # (walrus requires a single free dim on the weights AP);
            # per-partition counts (st marginal partials) via accum_out
            toh = pool_toh.tile([P, NT * CF], f16, tag="toh")
            toh_i = toh[:].rearrange("p (x i) -> p i x", i=NT)
            for i in range(1, N):
                nc.vector.tensor_scalar(
                    toh_i[:, i - 1, :], tm16[:], float(i), None, eq,
                    op1=add,
                    accum_out=stacc[:, (i - 1) * NCHUNK + c:(i - 1) * NCHUNK + c + 1],
                )
            # moving tile: 31 pred one-hots + cls + ln(cls) + ln(1-cls)
            mov = pool_mov.tile([P, MCOL * CF], f16, tag="mov")
            for k in range(KB):
                nc.vector.tensor_scalar(
                    mov[:, k * CF:(k + 1) * CF], pm16[:], float(k), None, eq,
                    op1=add,
                    accum_out=spacc[:, k * NCHUNK + c:k * NCHUNK + c + 1],
                )
            nc.scalar.activation(
                mov[:, KB * CF:(KB + 1) * CF], cls_t[:], AF.Copy,
                accum_out=auxacc[:, 0 * NCHUNK + c:0 * NCHUNK + c + 1],
            )
            nc.scalar.activation(
                mov[:, (KB + 1) * CF:(KB + 2) * CF], cls_t[:], AF.Ln,
                accum_out=auxacc[:, 1 * NCHUNK + c:1 * NCHUNK + c + 1],
            )
            nc.scalar.activation(
                mov[:, (KB + 2) * CF:(KB + 3) * CF], cls_t[:], AF.Ln,
                bias=1.0, scale=-1.0,
                accum_out=auxacc[:, 2 * NCHUNK + c:2 * NCHUNK + c + 1],
            )
            scr = pool_scr.tile([P, CF], f16, tag="scr")
            nc.scalar.activation(
                scr[:], ps16[:], AF.Square,
                accum_out=auxacc[:, 3 * NCHUNK + c:3 * NCHUNK + c + 1],
            )

            toh3 = toh[:].rearrange("p (g t) -> p g t", t=NT * J)
            mov3 = mov[:].rearrange("p (m x) -> p x m", m=MCOL)
            for g in range(CF // J):
                first = (c == 0 and g == 0)
                last = (c == NCHUNK - 1 and g == CF // J - 1)
                nc.tensor.matmul(
                    pair_ps[:, :],
                    toh3[:, g, :],
                    mov3[:, g * J:(g + 1) * J, :],
                    start=first,
                    stop=last,
                )
            off += CF

        pair_sb = pool_misc.tile([SROW, SCOL], f32)
        nc.scalar.copy(pair_sb[:], pair_ps[:])
        nc.scalar.dma_start(out=out_d[0:SROW, OC_PAIR:OC_PAIR + SCOL], in_=pair_sb[:])
        nc.scalar.dma_start(out=out_d[:, OC_SP:OC_SP + KB * NCHUNK], in_=spacc[:])
        nc.scalar.dma_start(out=out_d[:, OC_ST:OC_ST + NT * NCHUNK], in_=stacc[:])
        nc.scalar.dma_start(out=out_d[:, OC_AUX:OC_AUX + 4 * NCHUNK], in_=auxacc[:])

    nc.finalize()
    return nc


def _get_nc():
    if "nc" not in _cached:
        _cached["nc"] = _build_bass()
    return _cached["nc"]


def _get_runner():
    """Build the sharded jitted executable ONCE; reuse across calls.

    Mirrors concourse.bass2jax.run_bass_via_pjrt's multi-core path, but caches
    the jitted function so repeat calls skip retrace/recompile.
    """
    if "runner" in _cached:
        return _cached["runner"]

    import jax
    import concourse.mybir as mybir
    from jax.sharding import Mesh, PartitionSpec
    from jax.experimental.shard_map import shard_map
    from concourse import bass2jax

    bass2jax.install_neuronx_cc_hook()
    nc = _get_nc()
    partition_name = (
        nc.partition_id_tensor.name if nc.partition_id_tensor else None
    )

    in_names, out_names, out_avals, zero_outs = [], [], [], []
    for alloc in nc.m.functions[0].allocations:
        if not isinstance(alloc, mybir.MemoryLocationSet):
            continue
        name = alloc.memorylocations[0].name
        if alloc.kind == "ExternalInput":
            if name != partition_name:
                in_names.append(name)
        elif alloc.kind == "ExternalOutput":
            out_names.append(name)
            shape = tuple(alloc.tensor_shape)
            dtype = mybir.dt.np(alloc.dtype)
            out_avals.append(jax.core.ShapedArray(shape, dtype))
            zero_outs.append(np.zeros(shape, dtype))
    n_params = len(in_names)
    n_outs = len(out_avals)
    all_in_names = list(in_names) + list(out_names)
    if partition_name is not None:
        all_in_names.append(partition_name)
    donate = tuple(range(n_params, n_params + n_outs))

    def _body(*args):
        operands = list(args)
        if partition_name is not None:
            operands.append(bass2jax.partition_id_tensor())
        outs = bass2jax._bass_exec_p.bind(
            *operands,
            out_avals=tuple(out_avals),
            in_names=tuple(all_in_names),
            out_names=tuple(out_names),
            lowering_input_output_aliases=(),
            sim_require_finite=True,
            sim_require_nnan=True,
            nc=nc,
        )
        return tuple(outs)

    devices = jax.devices()[:NCORES]
    mesh = Mesh(np.asarray(devices), ("core",))
    in_specs = (PartitionSpec("core"),) * (n_params + n_outs)
    out_specs = (PartitionSpec("core"),) * n_outs
    sharded = jax.jit(
        shard_map(
            _body, mesh=mesh, in_specs=in_specs, out_specs=out_specs,
            check_rep=False,
        ),
        donate_argnums=donate,
        keep_unused=True,
    )

    def run(in_maps):
        concat_in = [
            np.concatenate([np.asarray(m[name]) for m in in_maps], axis=0)
            for name in in_names
        ]
        concat_zeros = [
            np.zeros((NCORES * z.shape[0], *z.shape[1:]), z.dtype)
            for z in zero_outs
        ]
        out_arrs = sharded(*concat_in, *concat_zeros)
        return [
            {
                name: np.asarray(out_arrs[i]).reshape(
                    NCORES, *out_avals[i].shape)[c]
                for i, name in enumerate(out_names)
            }
            for c in range(NCORES)
        ]

    def bench(in_maps, iters=20):
        """Time the sharded call with device-resident inputs."""
        import time
        from jax.sharding import NamedSharding

        concat_in = [
            np.concatenate([np.asarray(m[name]) for m in in_maps], axis=0)
            for name in in_names
        ]
        shard = NamedSharding(mesh, PartitionSpec("core"))
        dev_in = [jax.device_put(x, shard) for x in concat_in]
        zeros = [
            np.zeros((NCORES * z.shape[0], *z.shape[1:]), z.dtype)
            for z in zero_outs
        ]

        def call():
            zs = [jax.device_put(z, shard) for z in zeros]
            outs = sharded(*dev_in, *zs)
            for o in outs:
                o.block_until_ready()

        call()
        ts = []
        for _ in range(iters):
            t0 = time.perf_counter()
            call()
            ts.append(time.perf_counter() - t0)
        return min(ts), sum(ts) / len(ts)

    run.bench = bench
    _cached["runner"] = run
    return run


def _in_maps_for(pred_instance_mask, pred_score, cls_out, target_mask):
    in_maps = []
    for c in range(NCORES):
        rs = slice(c * ROWS, (c + 1) * ROWS)
        in_maps.append({
            "pm": np.ascontiguousarray(
                pred_instance_mask[rs]).reshape(-1).astype(np.int32),
            "tm": np.ascontiguousarray(
                target_mask[rs]).reshape(-1).astype(np.int32),
            "cls": np.ascontiguousarray(
                cls_out[rs]).reshape(-1).astype(np.float32),
            "ps": np.ascontiguousarray(
                pred_score[rs]).reshape(-1).astype(np.float32),
        })
    return in_maps


def kernel(pred_instance_mask, pred_score, cls_out, target_mask):
    run = _get_runner()
    in_maps = _in_maps_for(pred_instance_mask, pred_score, cls_out, target_mask)
    outs = [r["out"] for r in run(in_maps)]
    return _host_finish(outs)


def _host_finish(outs):
    M = np.zeros((NT, MCOL), dtype=np.float64)     # rows n=1..15
    sp_part = np.zeros(KB, dtype=np.float64)       # pred marginals k=0..30
    st_part = np.zeros(NT, dtype=np.float64)       # target marginals n=1..15
    aux = np.zeros(4, dtype=np.float64)            # sum cls, ln, ln1m, ps^2
    for o in outs:
        o = o.astype(np.float64)
        pair = o[0:SROW, OC_PAIR:OC_PAIR + SCOL].reshape(J, NT, J, MCOL)
        M += pair[np.arange(J), :, np.arange(J), :].sum(axis=0)
        sp_part += o[:, OC_SP:OC_SP + KB * NCHUNK].reshape(P, KB, NCHUNK).sum(axis=(0, 2))
        st_part += o[:, OC_ST:OC_ST + NT * NCHUNK].reshape(P, NT, NCHUNK).sum(axis=(0, 2))
        aux += o[:, OC_AUX:OC_AUX + 4 * NCHUNK].reshape(P, 4, NCHUNK).sum(axis=(0, 2))

    # reconstruct the full [N, K] joint histogram
    inter = np.zeros((N, K), dtype=np.float64)
    inter[1:, :KB] = M[:, :KB]
    st = np.zeros(N, dtype=np.float64)
    st[1:] = st_part
    st[0] = HWPIX - st_part.sum()
    sp = np.zeros(K, dtype=np.float64)
    sp[:KB] = sp_part
    sp[KB] = HWPIX - sp_part.sum()
    inter[1:, KB] = st[1:] - M[:, :KB].sum(axis=1)
    inter[0, :KB] = sp[:KB] - inter[1:, :KB].sum(axis=0)
    inter[0, KB] = st[0] - inter[0, :KB].sum()

    sum_t = HWPIX - st[0]             # count(target > 0)
    sum_p = aux[0]                    # sum(cls_out)
    inter_cls = M[:, KB].sum()        # sum over target>0 of cls_out
    bce_sum = M[:, KB + 1].sum() + (aux[2] - M[:, KB + 2].sum())

    mse = aux[3] / HWPIX
    bce_cls = -bce_sum / HWPIX
    dice_cls = 1.0 - (2.0 * inter_cls + SMOOTH) / (sum_p + sum_t + SMOOTH)

    union = st[:, None] + sp[None, :]
    bce_pair = 100.0 * (union - 2.0 * inter) / HWPIX
    dice_pair = 1.0 - (2.0 * inter + SMOOTH) / (union + SMOOTH)
    pair = bce_pair + dice_pair
    res = mse + bce_cls + dice_cls + pair.min(axis=1).sum()
    return np.float32(res / float(N))
